# revision 118
# baseline (speedup 1.0000x reference)
"""GQA (grouped-query attention) Trainium2 Bass kernel, v2.

Problem: B=4, T=2048, E=1536, 8 kv-groups; per group one attention head of
dim D=192 (q projected to 192; k/v projected to 64 and channel-tiled 3x),
interleaved-pair RoPE on q and tiled-k, causal softmax, out = P @ v_tiled.

Key algebraic facts exploited (carried over from v1):
  * Channel permutations applied identically to q and k leave scores
    unchanged -> host permutes Wq columns to rotate-half order (reals then
    imags) so RoPE on device is 6 slice-wise vector ops.
  * k_tiled's 3 copies see *different* RoPE angles; with the rotate-half
    storage each of the 96 pair-rows reads base channel (j mod 32) of the
    even/odd-reordered 64-dim k -> built with stride-0 repeat APs.
  * v is NOT roped, so out channels repeat exactly 3x within each group:
    only P @ v64 (64 cols + 1 ones-col for the softmax denominator) is
    computed; the DMA to HBM replicates it 3x with a stride-0 source AP.
  * No max subtraction needed (|scores*scale| < ~6 for this data).

New in v2:
  * Host supplies x already transposed AND cast to bf16 ("xt" [E, T]):
    the projection's stationary operand is xt chunks directly -- the 384
    PE x-transposes and their PSUM->SBUF copies are gone.
  * Whole q/k pipeline in bf16: weights, rope tables, roped q/k, P, v.
    PE transposes of roped q/k run at 1 cyc/row (vs 2 for fp32), DVE rope
    runs in 2x mode, weight/x DMA halves.
  * Causal subranges: for a diagonal S block (k-chunk kc inside q-chunk
    qc), only q-columns >= 128*d (d = kc - 4*qc) are computed -- the S
    matmuls, exp, and PV matmuls all shrink their free range. Saves ~25%
    of S+PV+exp work; the memset of masked pT cols is gone too.

Dataflow (per core): one batch b = core//2, four groups gh = core%2,
2 passes x 2 groups. S^T layout flash attention as v1.

Sharding: 8 cores = 4 batches x 2 group-halves; each core writes its
(T, 768) slice; host reassembles (B, T, 1536).
"""

import math
from contextlib import ExitStack

import numpy as np

import concourse.bass as bass
import concourse.mybir as mybir
import concourse.tile as tile
from concourse import bacc
from concourse.bass_utils import run_bass_kernel_spmd
from concourse.masks import make_identity

B, T, E = 4, 2048, 1536
G = 8            # kv heads (groups)
HD = 64          # per-head dim of k/v before tiling
REP = 3
D = REP * HD     # 192, per-group attention dim
P = 128
NT = T // P      # 16 row tiles
NE = E // P      # 12 contraction chunks
GPC = 4          # groups per core
NPASS = 2        # projection passes per core
GPP = GPC // NPASS  # groups per pass
WBLK = GPP * D + GPP * HD + GPP * HD   # 640 weight cols per pass
WCOLS = NPASS * WBLK                   # 1280
QKW = GPP * D + GPP * HD               # 512: q+k cols per pass
THETA = 10000.0
SCALE = 1.0 / math.sqrt(D)
QCH = 512        # q chunk (matmul free dim / PSUM bank)
NQC = T // QCH   # 4
NKC = T // P     # 16 k chunks
DPQ = QCH // P   # 4 diagonal k-blocks per q chunk
SEAM_FILL = 10   # SDPA blocks emitted before the final rope-tail drains

F32 = mybir.dt.float32
BF16 = mybir.dt.bfloat16
WARMUP = 113     # dummy PE transposes at t=0 (p-state ramp + DMA-wait fill)


def _build_nc(use_bias=False):
    nc = bacc.Bacc("TRN2", target_bir_lowering=False, debug=False)

    xt_d = nc.dram_tensor("xt", [E, T], BF16, kind="ExternalInput").ap()
    w_d = nc.dram_tensor("w", [E, WCOLS], BF16, kind="ExternalInput").ap()
    b_d = nc.dram_tensor("bias", [1, WCOLS], BF16, kind="ExternalInput").ap()
    cos_d = nc.dram_tensor("cos", [T, D // 2], BF16, kind="ExternalInput").ap()
    sin_d = nc.dram_tensor("sin", [T, D // 2], BF16, kind="ExternalInput").ap()
    out_d = nc.dram_tensor("out", [T, GPC * D], BF16,
                           kind="ExternalOutput").ap()

    mult = mybir.AluOpType.mult

    with tile.TileContext(nc) as tc, ExitStack() as ctx:
        singles = ctx.enter_context(tc.tile_pool(name="singles", bufs=1))
        qkv_pool = ctx.enter_context(tc.tile_pool(name="qkv", bufs=2))
        small = ctx.enter_context(tc.tile_pool(name="small", bufs=3))
        ppool = ctx.enter_context(tc.tile_pool(name="ppool", bufs=6))
        opool = ctx.enter_context(tc.tile_pool(name="opool", bufs=2))
        ps_proj = ctx.enter_context(tc.tile_pool(name="ps_proj", bufs=2, space="PSUM"))
        ps_v = ctx.enter_context(tc.tile_pool(name="ps_v", bufs=1, space="PSUM"))
        ps_t = ctx.enter_context(tc.tile_pool(name="ps_t", bufs=2, space="PSUM"))
        ps_s = ctx.enter_context(tc.tile_pool(name="ps_s", bufs=2, space="PSUM"))
        ps_o = ctx.enter_context(tc.tile_pool(name="ps_o", bufs=1, space="PSUM"))

        ident = singles.tile([P, P], BF16)
        make_identity(nc, ident)
        # causal triangle mask: tri[p, f] = 1.0 if f >= p else 0
        tri = singles.tile([P, P], BF16, name="tri", tag="tri")
        nc.gpsimd.memset(tri, 1.0)
        nc.gpsimd.affine_select(
            out=tri, in_=tri, pattern=[[1, P]],
            compare_op=mybir.AluOpType.is_ge, fill=0.0,
            base=0, channel_multiplier=-1)

        # PE warm-up: chained dummy matmuls on never-written SBUF keep the PE
        # busy while the first DMAs land, so real matmuls start at full clock
        # (the p-state ramp needs ~3us of continuous PE activity).  Results
        # land in a PSUM bank that is immediately recycled.
        junk = singles.tile([P, P], BF16, name="junk", tag="junk")
        nc.vector.memset(junk[:, 0:1], 0.0)
        warm = ps_t.tile([P, 4 * P], BF16, tag="tps", name="warm")
        for _ in range(WARMUP):
            nc.tensor.transpose(warm[:, 0:P], junk, junk)

        # weights: [128, NE, WCOLS] bf16.  DMA order is tuned so the first
        # projection tile's operands land ASAP: pass-0 weights first, then
        # ascending x^T column blocks just-in-time for the ti loop.
        w_sb = singles.tile([P, NE, WCOLS], BF16)
        w_r = w_d.rearrange("(eo p) c -> p eo c", p=P)
        xt_sb = singles.tile([P, NE, T], BF16)
        xt_r = xt_d.rearrange("(eo p) t -> p eo t", p=P)
        cos_sb = singles.tile([P, NT, D // 2], BF16)
        sin_sb = singles.tile([P, NT, D // 2], BF16)
        # issue order tuned against the serial DMA pool: pass-0 weights,
        # then xt col-blocks ascending, rope tables interleaved
        nc.sync.dma_start(w_sb[:, 0:6, 0:WBLK], w_r[:, 0:6, 0:WBLK])
        nc.sync.dma_start(w_sb[:, 6:NE, 0:WBLK], w_r[:, 6:NE, 0:WBLK])
        nc.scalar.dma_start(xt_sb[:, :, 0:256], xt_r[:, :, 0:256])
        cos_r = cos_d.rearrange("(n p) c -> p n c", p=P)
        sin_r = sin_d.rearrange("(n p) c -> p n c", p=P)
        nc.gpsimd.dma_start(cos_sb[:, 0:4, :], cos_r[:, 0:4, :])
        nc.gpsimd.dma_start(sin_sb[:, 0:4, :], sin_r[:, 0:4, :])
        nc.scalar.dma_start(xt_sb[:, :, 256:512], xt_r[:, :, 256:512])
        nc.gpsimd.dma_start(cos_sb[:, 4:NT, :], cos_r[:, 4:NT, :])
        nc.gpsimd.dma_start(sin_sb[:, 4:NT, :], sin_r[:, 4:NT, :])
        xt_eng = [None, None, nc.sync, nc.scalar,
                  nc.sync, nc.scalar, nc.sync, nc.scalar]
        for bi in range(2, 8):
            lo, hi = bi * 256, (bi + 1) * 256
            xt_eng[bi].dma_start(xt_sb[:, :, lo:hi], xt_r[:, :, lo:hi])
        nc.scalar.dma_start(w_sb[:, 0:6, WBLK:WCOLS], w_r[:, 0:6, WBLK:WCOLS])
        nc.sync.dma_start(w_sb[:, 6:NE, WBLK:WCOLS], w_r[:, 6:NE, WBLK:WCOLS])
        if use_bias:
            b_sb = singles.tile([1, WCOLS], BF16)
            nc.sync.dma_start(b_sb, b_d)
            ones = singles.tile([1, P], BF16)
            nc.gpsimd.memset(ones, 1.0)

        for h in range(NPASS):
            woff = h * WBLK
            qT_hi = qkv_pool.tile([P, GPP, T], BF16, tag="qT_hi", name="qT_hi")
            qT_lo = qkv_pool.tile([D - P, GPP, T], BF16, tag="qT_lo", name="qT_lo")
            kT_hi = qkv_pool.tile([P, GPP, T], BF16, tag="kT_hi", name="kT_hi")
            kT_lo = qkv_pool.tile([D - P, GPP, T], BF16, tag="kT_lo", name="kT_lo")
            v_sb = qkv_pool.tile([P, NT, GPP, HD + 1], BF16, tag="v_sb",
                                 name="v_sb")
            nc.gpsimd.memset(v_sb[:, :, :, HD:HD + 1], 1.0)

            # ---- per-tile rope+transpose, pipelined 2 tiles behind proj ----
            def emit_rope(ti, stage, qT_hi=qT_hi, qT_lo=qT_lo, kT_hi=kT_hi,
                          kT_lo=kT_lo):
                cosv = cos_sb[:, ti, :]
                sinv = sin_sb[:, ti, :]
                roped = small.tile([P, 2 * GPP * D], BF16, tag="roped",
                                   name="roped")
                # --- q rope, both groups (rotate-half layout) ---
                qv = stage[:, 0:GPP * D].rearrange("p (g d) -> p g d", g=GPP)
                qR = qv[:, :, 0:D // 2]
                qI = qv[:, :, D // 2:D]
                cosb = cosv[:, None, :].to_broadcast((P, GPP, D // 2))
                sinb = sinv[:, None, :].to_broadcast((P, GPP, D // 2))
                qo = roped[:, 0:GPP * D].rearrange("p (g d) -> p g d", g=GPP)
                qo0 = qo[:, :, 0:D // 2]
                qo1 = qo[:, :, D // 2:D]
                tmp = small.tile([P, GPP * (D // 2)], BF16, tag="ropetmp",
                                 name="ropetmp")
                tmpg = tmp.rearrange("p (g d) -> p g d", g=GPP)
                nc.vector.tensor_tensor(qo0, qR, cosb, mult)
                nc.vector.tensor_tensor(tmpg, qI, sinb, mult)
                nc.vector.tensor_sub(qo0, qo0, tmpg)
                nc.vector.tensor_tensor(qo1, qR, sinb, mult)
                nc.vector.tensor_tensor(tmpg, qI, cosb, mult)
                nc.vector.tensor_add(qo1, qo1, tmpg)

                # --- k: expand 64 -> 192 with per-copy rope, both groups ---
                kv = stage[:, GPP * D:QKW].rearrange("p (g c) -> p g c", g=GPP)
                kR = kv[:, :, None, 0:32].to_broadcast((P, GPP, REP, 32))
                kI = kv[:, :, None, 32:HD].to_broadcast((P, GPP, REP, 32))
                cos3 = cosv.rearrange("p (r c) -> p r c", r=REP)
                sin3 = sinv.rearrange("p (r c) -> p r c", r=REP)
                cos3b = cos3[:, None, :, :].to_broadcast((P, GPP, REP, 32))
                sin3b = sin3[:, None, :, :].to_broadcast((P, GPP, REP, 32))
                ko = roped[:, GPP * D:2 * GPP * D].rearrange(
                    "p (g u r c) -> p g u r c", g=GPP, u=2, r=REP)
                ko0 = ko[:, :, 0]
                ko1 = ko[:, :, 1]
                tmp3 = tmpg.rearrange("p g (r c) -> p g r c", r=REP)
                nc.vector.tensor_tensor(ko0, kR, cos3b, mult)
                nc.vector.tensor_tensor(tmp3, kI, sin3b, mult)
                nc.vector.tensor_sub(ko0, ko0, tmp3)
                nc.vector.tensor_tensor(ko1, kR, sin3b, mult)
                nc.vector.tensor_tensor(tmp3, kI, cos3b, mult)
                nc.vector.tensor_add(ko1, ko1, tmp3)

                # --- transposes (bf16, 1 cyc/row) into bf16 PSUM banks ---
                # bank layout: cols 0:128 hi-g0, 128:256 hi-g1,
                #              256:384 lo-g0 (parts 0:64), 384:512 lo-g1
                tq = ps_t.tile([P, 4 * P], BF16, tag="tps", name="tq")
                for g in range(GPP):
                    nc.tensor.transpose(tq[:, g * P:(g + 1) * P],
                                        roped[:, g * D:g * D + P], ident)
                    nc.tensor.transpose(
                        tq[0:D - P, (GPP + g) * P:(GPP + g + 1) * P],
                        roped[:, g * D + P:(g + 1) * D], ident)
                nc.scalar.copy(
                    qT_hi[:, :, ti * P:(ti + 1) * P],
                    tq[:, 0:GPP * P].rearrange("p (g t) -> p g t", g=GPP))
                nc.scalar.copy(
                    qT_lo[:, :, ti * P:(ti + 1) * P],
                    tq[0:D - P, GPP * P:2 * GPP * P].rearrange(
                        "p (g t) -> p g t", g=GPP))
                tk = ps_t.tile([P, 4 * P], BF16, tag="tps", name="tk")
                kb = GPP * D
                for g in range(GPP):
                    nc.tensor.transpose(tk[:, g * P:(g + 1) * P],
                                        roped[:, kb + g * D:kb + g * D + P],
                                        ident)
                    nc.tensor.transpose(
                        tk[0:D - P, (GPP + g) * P:(GPP + g + 1) * P],
                        roped[:, kb + g * D + P:kb + (g + 1) * D], ident)
                nc.vector.tensor_copy(
                    kT_hi[:, :, ti * P:(ti + 1) * P],
                    tk[:, 0:GPP * P].rearrange("p (g t) -> p g t", g=GPP))
                nc.vector.tensor_copy(
                    kT_lo[:, :, ti * P:(ti + 1) * P],
                    tk[0:D - P, GPP * P:2 * GPP * P].rearrange(
                        "p (g t) -> p g t", g=GPP))

            # ---- projection over row tiles ----
            pending = []
            pv = None
            for ti in range(NT):
                pqk = ps_proj.tile([P, QKW], F32, tag="pqk", name="pqk")
                if ti % 4 == 0:
                    pv = ps_v.tile([P, 4, GPP * HD], F32, tag="pv", name="pv")
                pvs = pv[:, ti % 4, :]
                # on bank-rotation tiles (ti%4==0) run all q/k matmuls
                # before the first v matmul: the fresh pv bank may still be
                # waiting on the previous 4-tile drain
                if ti % 4 == 0:
                    for eo in range(NE):
                        lhsT = xt_sb[:, eo, ti * P:(ti + 1) * P]
                        last = (eo == NE - 1) and not use_bias
                        nc.tensor.matmul(
                            pqk, lhsT, w_sb[:, eo, woff:woff + QKW],
                            start=(eo == 0), stop=last)
                    for eo in range(NE):
                        lhsT = xt_sb[:, eo, ti * P:(ti + 1) * P]
                        last = (eo == NE - 1) and not use_bias
                        nc.tensor.matmul(
                            pvs, lhsT, w_sb[:, eo, woff + QKW:woff + WBLK],
                            start=(eo == 0), stop=last)
                else:
                    for eo in range(NE):
                        lhsT = xt_sb[:, eo, ti * P:(ti + 1) * P]
                        last = (eo == NE - 1) and not use_bias
                        nc.tensor.matmul(
                            pqk, lhsT, w_sb[:, eo, woff:woff + QKW],
                            start=(eo == 0), stop=last)
                        nc.tensor.matmul(
                            pvs, lhsT, w_sb[:, eo, woff + QKW:woff + WBLK],
                            start=(eo == 0), stop=last)
                if use_bias:
                    nc.tensor.matmul(pqk, ones, b_sb[:, woff:woff + QKW],
                                     start=False, stop=True)
                    nc.tensor.matmul(pvs, ones,
                                     b_sb[:, woff + QKW:woff + WBLK],
                                     start=False, stop=True)
                if ti % 4 == 3:
                    # drain 4 tiles of v at once: [128, 4, GPP, HD]
                    t0 = ti - 3
                    nc.scalar.copy(
                        v_sb[:, t0:t0 + 4, :, 0:HD],
                        pv.rearrange("p tt (g c) -> p tt g c", g=GPP))
                # stage q/k out of PSUM right away (frees the bank); rope
                # lags 2 tiles so the in-order PE queue never blocks on DVE
                stage = small.tile([P, QKW], BF16, tag="stage", name="stage")
                nc.vector.tensor_copy(stage, pqk)
                pending.append((ti, stage))
                if len(pending) > 2:
                    emit_rope(*pending.pop(0))
            # The last 2 tiles' rope chains are still pending; the first
            # SDPA blocks only touch early q/k tiles, so emit those blocks
            # FIRST -- their S matmuls keep the PE busy while DVE drains the
            # rope tail (the in-order PE queue would otherwise stall on the
            # final transposes).

            # ---- SDPA, both groups as ONE flattened block stream so the
            # lookahead spans the group seam (next group's S matmuls fill
            # the previous group's exp-wait tail) ----
            if True:
                s_idx = [0]

                def emit_s(j, qc, kc):
                    d = kc - DPQ * qc
                    off = P * d if d > 0 else 0
                    # round-robin the score bank over ps_s AND the two
                    # projection pools (idle during SDPA): an effective
                    # 5-bank rotation, so S(i) no longer waits on
                    # exp(i-2)'s drain semaphore every other block
                    k = s_idx[0] % 5
                    s_idx[0] += 1
                    if k < 2:
                        s_ps = ps_s.tile([P, QCH], F32, tag="sps",
                                         name="sps")
                    elif k < 4:
                        s_ps = ps_proj.tile([P, QCH], F32, tag="pqk",
                                            name="sps")
                    else:
                        s_ps = ps_v.tile([P, QCH], F32, tag="pv",
                                         name="sps")
                    nc.tensor.matmul(
                        s_ps[:, off:QCH], kT_hi[:, j, kc * P:(kc + 1) * P],
                        qT_hi[:, j, qc * QCH + off:(qc + 1) * QCH],
                        start=True, stop=False)
                    nc.tensor.matmul(
                        s_ps[:, off:QCH], kT_lo[:, j, kc * P:(kc + 1) * P],
                        qT_lo[:, j, qc * QCH + off:(qc + 1) * QCH],
                        start=False, stop=True)
                    pT = ppool.tile([P, QCH], BF16, tag="pT", name="pT")
                    nc.scalar.activation(pT[:, off:QCH], s_ps[:, off:QCH],
                                         mybir.ActivationFunctionType.Exp,
                                         scale=SCALE)
                    if d >= 0:  # diagonal 128x128 block: causal zeroing
                        nc.vector.tensor_tensor(pT[:, off:off + P],
                                                pT[:, off:off + P],
                                                tri, mult)
                    return pT, off

                blocks = [(j, qc, kc) for j in range(GPP)
                          for qc in range(NQC)
                          for kc in range(DPQ * (qc + 1))]
                pTs = {}
                LOOKAHEAD = 4
                for i in range(LOOKAHEAD):
                    pTs[blocks[i]] = emit_s(*blocks[i])
                o_ps = None
                for i, (j, qc, kc) in enumerate(blocks):
                    lg = GPP * h + j
                    if i in (SEAM_FILL, SEAM_FILL + 4) and pending:
                        emit_rope(*pending.pop(0))
                    # deeper lookahead over the exp-starved final blocks
                    lk = LOOKAHEAD + (1 if i >= len(blocks) - 12 else 0)
                    for t in (i + LOOKAHEAD, i + lk):
                        if t < len(blocks) and blocks[t] not in pTs:
                            pTs[blocks[t]] = emit_s(*blocks[t])
                    kmax = DPQ * (qc + 1)
                    if kc == 0:
                        o_ps = ps_o.tile([HD + 1, QCH], F32, tag="ops",
                                         name="ops")
                    pT, off = pTs.pop((j, qc, kc))
                    nc.tensor.matmul(o_ps[:, off:QCH], v_sb[:, kc, j, :],
                                     pT[:, off:QCH],
                                     start=(kc == 0), stop=(kc == kmax - 1))
                    if kc != kmax - 1:
                        continue
                    # ---- finalize q-chunk qc (bf16, per-128-row pipeline) ----
                    last = (h == NPASS - 1 and i == len(blocks) - 1)
                    o_sb = opool.tile([HD + 1, QCH], BF16, tag="o_sb",
                                      name="o_sb")
                    nc.vector.tensor_copy(o_sb, o_ps)
                    NB = QCH // P
                    tpo = ps_t.tile([P, NB * (HD + 2)], BF16, tag="tps",
                                    name="tpo")
                    nat3 = opool.tile([P, NB, REP * HD], BF16, tag="nat",
                                      name="nat")
                    rec = opool.tile([P, NB], F32, tag="rec", name="rec")
                    for blk in range(NB):
                        nc.tensor.transpose(
                            tpo[:, blk * (HD + 2):blk * (HD + 2) + HD + 1],
                            o_sb[:, blk * P:(blk + 1) * P],
                            ident[0:HD + 1, 0:HD + 1])
                    dma_eng = ([nc.sync, nc.scalar] if last else
                               [nc.gpsimd, nc.sync])
                    # one strided reciprocal covers all 4 denominators; the
                    # normalize runs as two 2-block ops so each output DMA
                    # launches as soon as its half is ready
                    tpb = tpo.rearrange("p (b c) -> p b c", b=NB)
                    nc.vector.reciprocal(rec, tpb[:, :, HD])
                    for half in range(2):
                        b0 = 2 * half
                        nc.vector.tensor_tensor(
                            nat3[:, b0:b0 + 2, :].rearrange(
                                "p b (r c) -> p b r c", r=REP),
                            tpb[:, b0:b0 + 2, None, 0:HD].to_broadcast(
                                (P, 2, REP, HD)),
                            rec[:, b0:b0 + 2, None, None].to_broadcast(
                                (P, 2, REP, HD)), mult)
                        row0 = qc * QCH + b0 * P
                        dst = out_d[row0:row0 + 2 * P,
                                    lg * D:(lg + 1) * D].rearrange(
                            "(b t) c -> t b c", b=2)
                        dma_eng[half].dma_start(
                            dst, nat3[:, b0:b0 + 2, :])

    nc.compile()
    return nc


_NC_CACHE = {}


def _get_nc(use_bias=False):
    if use_bias not in _NC_CACHE:
        _NC_CACHE[use_bias] = _build_nc(use_bias)
    return _NC_CACHE[use_bias]


def _host_inputs(x, Wq, bq, Wk, bk, Wv, bv):
    import ml_dtypes
    bf16 = ml_dtypes.bfloat16

    j = np.arange(D // 2)
    angles = 1.0 / (THETA ** ((2.0 * j) / D))
    th = np.arange(T, dtype=np.float64)[:, None] * angles[None, :]
    cosn = np.cos(th).astype(bf16)
    sinn = np.sin(th).astype(bf16)

    perm_q = np.concatenate([np.arange(0, D, 2), np.arange(1, D, 2)])
    eo = np.concatenate([np.arange(0, HD, 2), np.arange(1, HD, 2)])

    Wq = np.asarray(Wq, np.float32)
    Wk = np.asarray(Wk, np.float32)
    Wv = np.asarray(Wv, np.float32)
    bq = np.asarray(bq, np.float32)
    bk = np.asarray(bk, np.float32)
    bv = np.asarray(bv, np.float32)
    x = np.asarray(x, np.float32)

    in_maps = []
    for c in range(8):
        b, gh = divmod(c, 2)
        wblocks, bblocks = [], []
        for hh in range(NPASS):
            gs = [gh * GPC + GPP * hh + jj for jj in range(GPP)]
            for g in gs:
                wblocks.append(Wq[:, g * D:(g + 1) * D][:, perm_q])
                bblocks.append(bq[g * D:(g + 1) * D][perm_q])
            for g in gs:
                wblocks.append(Wk[:, g * HD:(g + 1) * HD][:, eo])
                bblocks.append(bk[g * HD:(g + 1) * HD][eo])
            for g in gs:
                wblocks.append(Wv[:, g * HD:(g + 1) * HD])
                bblocks.append(bv[g * HD:(g + 1) * HD])
        w_core = np.ascontiguousarray(
            np.concatenate(wblocks, axis=1)).astype(bf16)
        b_core = np.concatenate(bblocks)[None, :].astype(bf16)
        b_core = np.ascontiguousarray(b_core)
        in_maps.append({
            "xt": np.ascontiguousarray(x[b].T).astype(bf16),
            "w": w_core,
            "bias": b_core,
            "cos": cosn,
            "sin": sinn,
        })
    return in_maps


def kernel(x, Wq, bq, Wk, bk, Wv, bv, _trace=False, _trace_kwargs=None):
    in_maps = _host_inputs(x, Wq, bq, Wk, bk, Wv, bv)
    use_bias = bool(max(np.abs(np.asarray(b)).max() for b in (bq, bk, bv)) > 0)
    nc = _get_nc(use_bias)
    res = run_bass_kernel_spmd(nc, in_maps, core_ids=list(range(8)),
                               trace=_trace, **(_trace_kwargs or {}))
    out = np.empty((B, T, E), np.float32)
    for c in range(8):
        b, gh = divmod(c, 2)
        out[b, :, gh * GPC * D:(gh + 1) * GPC * D] = \
            res.results[c]["out"].astype(np.float32)
    if _trace:
        return out, res
    return out


# revision 119
# speedup vs baseline: 1.0008x; 1.0008x over previous
"""GQA (grouped-query attention) Trainium2 Bass kernel, v2.

Problem: B=4, T=2048, E=1536, 8 kv-groups; per group one attention head of
dim D=192 (q projected to 192; k/v projected to 64 and channel-tiled 3x),
interleaved-pair RoPE on q and tiled-k, causal softmax, out = P @ v_tiled.

Key algebraic facts exploited (carried over from v1):
  * Channel permutations applied identically to q and k leave scores
    unchanged -> host permutes Wq columns to rotate-half order (reals then
    imags) so RoPE on device is 6 slice-wise vector ops.
  * k_tiled's 3 copies see *different* RoPE angles; with the rotate-half
    storage each of the 96 pair-rows reads base channel (j mod 32) of the
    even/odd-reordered 64-dim k -> built with stride-0 repeat APs.
  * v is NOT roped, so out channels repeat exactly 3x within each group:
    only P @ v64 (64 cols + 1 ones-col for the softmax denominator) is
    computed; the DMA to HBM replicates it 3x with a stride-0 source AP.
  * No max subtraction needed (|scores*scale| < ~6 for this data).

New in v2:
  * Host supplies x already transposed AND cast to bf16 ("xt" [E, T]):
    the projection's stationary operand is xt chunks directly -- the 384
    PE x-transposes and their PSUM->SBUF copies are gone.
  * Whole q/k pipeline in bf16: weights, rope tables, roped q/k, P, v.
    PE transposes of roped q/k run at 1 cyc/row (vs 2 for fp32), DVE rope
    runs in 2x mode, weight/x DMA halves.
  * Causal subranges: for a diagonal S block (k-chunk kc inside q-chunk
    qc), only q-columns >= 128*d (d = kc - 4*qc) are computed -- the S
    matmuls, exp, and PV matmuls all shrink their free range. Saves ~25%
    of S+PV+exp work; the memset of masked pT cols is gone too.

Dataflow (per core): one batch b = core//2, four groups gh = core%2,
2 passes x 2 groups. S^T layout flash attention as v1.

Sharding: 8 cores = 4 batches x 2 group-halves; each core writes its
(T, 768) slice; host reassembles (B, T, 1536).
"""

import math
from contextlib import ExitStack

import numpy as np

import concourse.bass as bass
import concourse.mybir as mybir
import concourse.tile as tile
from concourse import bacc
from concourse.bass_utils import run_bass_kernel_spmd
from concourse.masks import make_identity

B, T, E = 4, 2048, 1536
G = 8            # kv heads (groups)
HD = 64          # per-head dim of k/v before tiling
REP = 3
D = REP * HD     # 192, per-group attention dim
P = 128
NT = T // P      # 16 row tiles
NE = E // P      # 12 contraction chunks
GPC = 4          # groups per core
NPASS = 2        # projection passes per core
GPP = GPC // NPASS  # groups per pass
WBLK = GPP * D + GPP * HD + GPP * HD   # 640 weight cols per pass
WCOLS = NPASS * WBLK                   # 1280
QKW = GPP * D + GPP * HD               # 512: q+k cols per pass
THETA = 10000.0
SCALE = 1.0 / math.sqrt(D)
QCH = 512        # q chunk (matmul free dim / PSUM bank)
NQC = T // QCH   # 4
NKC = T // P     # 16 k chunks
DPQ = QCH // P   # 4 diagonal k-blocks per q chunk
SEAM_FILL = 10   # SDPA blocks emitted before the final rope-tail drains

F32 = mybir.dt.float32
BF16 = mybir.dt.bfloat16
WARMUP = 113     # dummy PE transposes at t=0 (p-state ramp + DMA-wait fill)


def _build_nc(use_bias=False):
    nc = bacc.Bacc("TRN2", target_bir_lowering=False, debug=False)

    xt_d = nc.dram_tensor("xt", [E, T], BF16, kind="ExternalInput").ap()
    w_d = nc.dram_tensor("w", [E, WCOLS], BF16, kind="ExternalInput").ap()
    b_d = nc.dram_tensor("bias", [1, WCOLS], BF16, kind="ExternalInput").ap()
    cos_d = nc.dram_tensor("cos", [T, D // 2], BF16, kind="ExternalInput").ap()
    sin_d = nc.dram_tensor("sin", [T, D // 2], BF16, kind="ExternalInput").ap()
    out_d = nc.dram_tensor("out", [T, GPC * D], BF16,
                           kind="ExternalOutput").ap()

    mult = mybir.AluOpType.mult

    with tile.TileContext(nc) as tc, ExitStack() as ctx:
        singles = ctx.enter_context(tc.tile_pool(name="singles", bufs=1))
        qkv_pool = ctx.enter_context(tc.tile_pool(name="qkv", bufs=2))
        small = ctx.enter_context(tc.tile_pool(name="small", bufs=3))
        ppool = ctx.enter_context(tc.tile_pool(name="ppool", bufs=6))
        opool = ctx.enter_context(tc.tile_pool(name="opool", bufs=2))
        ps_proj = ctx.enter_context(tc.tile_pool(name="ps_proj", bufs=2, space="PSUM"))
        ps_v = ctx.enter_context(tc.tile_pool(name="ps_v", bufs=1, space="PSUM"))
        ps_t = ctx.enter_context(tc.tile_pool(name="ps_t", bufs=2, space="PSUM"))
        ps_s = ctx.enter_context(tc.tile_pool(name="ps_s", bufs=2, space="PSUM"))
        ps_o = ctx.enter_context(tc.tile_pool(name="ps_o", bufs=1, space="PSUM"))

        ident = singles.tile([P, P], BF16)
        make_identity(nc, ident)
        # causal triangle mask: tri[p, f] = 1.0 if f >= p else 0
        tri = singles.tile([P, P], BF16, name="tri", tag="tri")
        nc.gpsimd.memset(tri, 1.0)
        nc.gpsimd.affine_select(
            out=tri, in_=tri, pattern=[[1, P]],
            compare_op=mybir.AluOpType.is_ge, fill=0.0,
            base=0, channel_multiplier=-1)

        # PE warm-up: chained dummy matmuls on never-written SBUF keep the PE
        # busy while the first DMAs land, so real matmuls start at full clock
        # (the p-state ramp needs ~3us of continuous PE activity).  Results
        # land in a PSUM bank that is immediately recycled.
        junk = singles.tile([P, P], BF16, name="junk", tag="junk")
        nc.vector.memset(junk[:, 0:1], 0.0)
        warm = ps_t.tile([P, 4 * P], BF16, tag="tps", name="warm")
        for _ in range(WARMUP):
            nc.tensor.transpose(warm[:, 0:P], junk, junk)

        # weights: [128, NE, WCOLS] bf16.  DMA order is tuned so the first
        # projection tile's operands land ASAP: pass-0 weights first, then
        # ascending x^T column blocks just-in-time for the ti loop.
        w_sb = singles.tile([P, NE, WCOLS], BF16)
        w_r = w_d.rearrange("(eo p) c -> p eo c", p=P)
        xt_sb = singles.tile([P, NE, T], BF16)
        xt_r = xt_d.rearrange("(eo p) t -> p eo t", p=P)
        cos_sb = singles.tile([P, NT, D // 2], BF16)
        sin_sb = singles.tile([P, NT, D // 2], BF16)
        # issue order tuned against the serial DMA pool: pass-0 weights,
        # then xt col-blocks ascending, rope tables interleaved
        nc.sync.dma_start(w_sb[:, 0:6, 0:WBLK], w_r[:, 0:6, 0:WBLK])
        nc.sync.dma_start(w_sb[:, 6:NE, 0:WBLK], w_r[:, 6:NE, 0:WBLK])
        nc.scalar.dma_start(xt_sb[:, :, 0:256], xt_r[:, :, 0:256])
        cos_r = cos_d.rearrange("(n p) c -> p n c", p=P)
        sin_r = sin_d.rearrange("(n p) c -> p n c", p=P)
        nc.gpsimd.dma_start(cos_sb[:, 0:4, :], cos_r[:, 0:4, :])
        nc.gpsimd.dma_start(sin_sb[:, 0:4, :], sin_r[:, 0:4, :])
        nc.scalar.dma_start(xt_sb[:, :, 256:512], xt_r[:, :, 256:512])
        nc.gpsimd.dma_start(cos_sb[:, 4:NT, :], cos_r[:, 4:NT, :])
        nc.gpsimd.dma_start(sin_sb[:, 4:NT, :], sin_r[:, 4:NT, :])
        xt_eng = [None, None, nc.sync, nc.scalar,
                  nc.sync, nc.scalar, nc.sync, nc.scalar]
        for bi in range(2, 8):
            lo, hi = bi * 256, (bi + 1) * 256
            xt_eng[bi].dma_start(xt_sb[:, :, lo:hi], xt_r[:, :, lo:hi])
        nc.scalar.dma_start(w_sb[:, 0:6, WBLK:WCOLS], w_r[:, 0:6, WBLK:WCOLS])
        nc.sync.dma_start(w_sb[:, 6:NE, WBLK:WCOLS], w_r[:, 6:NE, WBLK:WCOLS])
        if use_bias:
            b_sb = singles.tile([1, WCOLS], BF16)
            nc.sync.dma_start(b_sb, b_d)
            ones = singles.tile([1, P], BF16)
            nc.gpsimd.memset(ones, 1.0)

        for h in range(NPASS):
            woff = h * WBLK
            qT_hi = qkv_pool.tile([P, GPP, T], BF16, tag="qT_hi", name="qT_hi")
            qT_lo = qkv_pool.tile([D - P, GPP, T], BF16, tag="qT_lo", name="qT_lo")
            kT_hi = qkv_pool.tile([P, GPP, T], BF16, tag="kT_hi", name="kT_hi")
            kT_lo = qkv_pool.tile([D - P, GPP, T], BF16, tag="kT_lo", name="kT_lo")
            v_sb = qkv_pool.tile([P, NT, GPP, HD + 1], BF16, tag="v_sb",
                                 name="v_sb")
            nc.gpsimd.memset(v_sb[:, :, :, HD:HD + 1], 1.0)

            # ---- per-tile rope+transpose, pipelined 2 tiles behind proj ----
            def emit_rope(ti, stage, qT_hi=qT_hi, qT_lo=qT_lo, kT_hi=kT_hi,
                          kT_lo=kT_lo):
                cosv = cos_sb[:, ti, :]
                sinv = sin_sb[:, ti, :]
                roped = small.tile([P, 2 * GPP * D], BF16, tag="roped",
                                   name="roped")
                # --- q rope, both groups (rotate-half layout) ---
                qv = stage[:, 0:GPP * D].rearrange("p (g d) -> p g d", g=GPP)
                qR = qv[:, :, 0:D // 2]
                qI = qv[:, :, D // 2:D]
                cosb = cosv[:, None, :].to_broadcast((P, GPP, D // 2))
                sinb = sinv[:, None, :].to_broadcast((P, GPP, D // 2))
                qo = roped[:, 0:GPP * D].rearrange("p (g d) -> p g d", g=GPP)
                qo0 = qo[:, :, 0:D // 2]
                qo1 = qo[:, :, D // 2:D]
                tmp = small.tile([P, GPP * (D // 2)], BF16, tag="ropetmp",
                                 name="ropetmp")
                tmpg = tmp.rearrange("p (g d) -> p g d", g=GPP)
                nc.vector.tensor_tensor(qo0, qR, cosb, mult)
                nc.vector.tensor_tensor(tmpg, qI, sinb, mult)
                nc.vector.tensor_sub(qo0, qo0, tmpg)
                nc.vector.tensor_tensor(qo1, qR, sinb, mult)
                nc.vector.tensor_tensor(tmpg, qI, cosb, mult)
                nc.vector.tensor_add(qo1, qo1, tmpg)

                # --- k: expand 64 -> 192 with per-copy rope, both groups ---
                kv = stage[:, GPP * D:QKW].rearrange("p (g c) -> p g c", g=GPP)
                kR = kv[:, :, None, 0:32].to_broadcast((P, GPP, REP, 32))
                kI = kv[:, :, None, 32:HD].to_broadcast((P, GPP, REP, 32))
                cos3 = cosv.rearrange("p (r c) -> p r c", r=REP)
                sin3 = sinv.rearrange("p (r c) -> p r c", r=REP)
                cos3b = cos3[:, None, :, :].to_broadcast((P, GPP, REP, 32))
                sin3b = sin3[:, None, :, :].to_broadcast((P, GPP, REP, 32))
                ko = roped[:, GPP * D:2 * GPP * D].rearrange(
                    "p (g u r c) -> p g u r c", g=GPP, u=2, r=REP)
                ko0 = ko[:, :, 0]
                ko1 = ko[:, :, 1]
                tmp3 = tmpg.rearrange("p g (r c) -> p g r c", r=REP)
                nc.vector.tensor_tensor(ko0, kR, cos3b, mult)
                nc.vector.tensor_tensor(tmp3, kI, sin3b, mult)
                nc.vector.tensor_sub(ko0, ko0, tmp3)
                nc.vector.tensor_tensor(ko1, kR, sin3b, mult)
                nc.vector.tensor_tensor(tmp3, kI, cos3b, mult)
                nc.vector.tensor_add(ko1, ko1, tmp3)

                # --- transposes (bf16, 1 cyc/row) into bf16 PSUM banks ---
                # bank layout: cols 0:128 hi-g0, 128:256 hi-g1,
                #              256:384 lo-g0 (parts 0:64), 384:512 lo-g1
                tq = ps_t.tile([P, 4 * P], BF16, tag="tps", name="tq")
                for g in range(GPP):
                    nc.tensor.transpose(tq[:, g * P:(g + 1) * P],
                                        roped[:, g * D:g * D + P], ident)
                    nc.tensor.transpose(
                        tq[0:D - P, (GPP + g) * P:(GPP + g + 1) * P],
                        roped[:, g * D + P:(g + 1) * D], ident)
                nc.scalar.copy(
                    qT_hi[:, :, ti * P:(ti + 1) * P],
                    tq[:, 0:GPP * P].rearrange("p (g t) -> p g t", g=GPP))
                nc.scalar.copy(
                    qT_lo[:, :, ti * P:(ti + 1) * P],
                    tq[0:D - P, GPP * P:2 * GPP * P].rearrange(
                        "p (g t) -> p g t", g=GPP))
                tk = ps_t.tile([P, 4 * P], BF16, tag="tps", name="tk")
                kb = GPP * D
                for g in range(GPP):
                    nc.tensor.transpose(tk[:, g * P:(g + 1) * P],
                                        roped[:, kb + g * D:kb + g * D + P],
                                        ident)
                    nc.tensor.transpose(
                        tk[0:D - P, (GPP + g) * P:(GPP + g + 1) * P],
                        roped[:, kb + g * D + P:kb + (g + 1) * D], ident)
                nc.vector.tensor_copy(
                    kT_hi[:, :, ti * P:(ti + 1) * P],
                    tk[:, 0:GPP * P].rearrange("p (g t) -> p g t", g=GPP))
                nc.vector.tensor_copy(
                    kT_lo[:, :, ti * P:(ti + 1) * P],
                    tk[0:D - P, GPP * P:2 * GPP * P].rearrange(
                        "p (g t) -> p g t", g=GPP))

            # ---- projection over row tiles ----
            pending = []
            pv = None
            for ti in range(NT):
                pqk = ps_proj.tile([P, QKW], F32, tag="pqk", name="pqk")
                if ti % 4 == 0:
                    pv = ps_v.tile([P, 4, GPP * HD], F32, tag="pv", name="pv")
                pvs = pv[:, ti % 4, :]
                # on bank-rotation tiles (ti%4==0) run all q/k matmuls
                # before the first v matmul: the fresh pv bank may still be
                # waiting on the previous 4-tile drain
                if ti % 4 == 0:
                    for eo in range(NE):
                        lhsT = xt_sb[:, eo, ti * P:(ti + 1) * P]
                        last = (eo == NE - 1) and not use_bias
                        nc.tensor.matmul(
                            pqk, lhsT, w_sb[:, eo, woff:woff + QKW],
                            start=(eo == 0), stop=last)
                    for eo in range(NE):
                        lhsT = xt_sb[:, eo, ti * P:(ti + 1) * P]
                        last = (eo == NE - 1) and not use_bias
                        nc.tensor.matmul(
                            pvs, lhsT, w_sb[:, eo, woff + QKW:woff + WBLK],
                            start=(eo == 0), stop=last)
                else:
                    for eo in range(NE):
                        lhsT = xt_sb[:, eo, ti * P:(ti + 1) * P]
                        last = (eo == NE - 1) and not use_bias
                        nc.tensor.matmul(
                            pqk, lhsT, w_sb[:, eo, woff:woff + QKW],
                            start=(eo == 0), stop=last)
                        nc.tensor.matmul(
                            pvs, lhsT, w_sb[:, eo, woff + QKW:woff + WBLK],
                            start=(eo == 0), stop=last)
                if use_bias:
                    nc.tensor.matmul(pqk, ones, b_sb[:, woff:woff + QKW],
                                     start=False, stop=True)
                    nc.tensor.matmul(pvs, ones,
                                     b_sb[:, woff + QKW:woff + WBLK],
                                     start=False, stop=True)
                if ti % 4 == 3:
                    # drain 4 tiles of v at once: [128, 4, GPP, HD]
                    t0 = ti - 3
                    nc.scalar.copy(
                        v_sb[:, t0:t0 + 4, :, 0:HD],
                        pv.rearrange("p tt (g c) -> p tt g c", g=GPP))
                # stage q/k out of PSUM right away (frees the bank); rope
                # lags 2 tiles so the in-order PE queue never blocks on DVE
                stage = small.tile([P, QKW], BF16, tag="stage", name="stage")
                nc.vector.tensor_copy(stage, pqk)
                pending.append((ti, stage))
                if len(pending) > 2:
                    emit_rope(*pending.pop(0))
            # The last 2 tiles' rope chains are still pending; the first
            # SDPA blocks only touch early q/k tiles, so emit those blocks
            # FIRST -- their S matmuls keep the PE busy while DVE drains the
            # rope tail (the in-order PE queue would otherwise stall on the
            # final transposes).

            # ---- SDPA, both groups as ONE flattened block stream so the
            # lookahead spans the group seam (next group's S matmuls fill
            # the previous group's exp-wait tail) ----
            if True:
                s_idx = [0]

                def emit_s(j, qc, kc):
                    d = kc - DPQ * qc
                    off = P * d if d > 0 else 0
                    # round-robin the score bank over ps_s AND the two
                    # projection pools (idle during SDPA): an effective
                    # 5-bank rotation, so S(i) no longer waits on
                    # exp(i-2)'s drain semaphore every other block
                    k = s_idx[0] % 5
                    s_idx[0] += 1
                    if k < 2:
                        s_ps = ps_s.tile([P, QCH], F32, tag="sps",
                                         name="sps")
                    elif k < 4:
                        s_ps = ps_proj.tile([P, QCH], F32, tag="pqk",
                                            name="sps")
                    else:
                        s_ps = ps_v.tile([P, QCH], F32, tag="pv",
                                         name="sps")
                    nc.tensor.matmul(
                        s_ps[:, off:QCH], kT_hi[:, j, kc * P:(kc + 1) * P],
                        qT_hi[:, j, qc * QCH + off:(qc + 1) * QCH],
                        start=True, stop=False)
                    nc.tensor.matmul(
                        s_ps[:, off:QCH], kT_lo[:, j, kc * P:(kc + 1) * P],
                        qT_lo[:, j, qc * QCH + off:(qc + 1) * QCH],
                        start=False, stop=True)
                    pT = ppool.tile([P, QCH], BF16, tag="pT", name="pT")
                    nc.scalar.activation(pT[:, off:QCH], s_ps[:, off:QCH],
                                         mybir.ActivationFunctionType.Exp,
                                         scale=SCALE)
                    if d >= 0:  # diagonal 128x128 block: causal zeroing
                        nc.vector.tensor_tensor(pT[:, off:off + P],
                                                pT[:, off:off + P],
                                                tri, mult)
                    return pT, off

                blocks = [(j, qc, kc) for j in range(GPP)
                          for qc in range(NQC)
                          for kc in range(DPQ * (qc + 1))]
                pTs = {}
                LOOKAHEAD = 4
                for i in range(LOOKAHEAD):
                    pTs[blocks[i]] = emit_s(*blocks[i])
                o_ps = None
                for i, (j, qc, kc) in enumerate(blocks):
                    lg = GPP * h + j
                    if i in (SEAM_FILL, SEAM_FILL + 4) and pending:
                        emit_rope(*pending.pop(0))
                    kmax = DPQ * (qc + 1)
                    if kc == 0:
                        o_ps = ps_o.tile([HD + 1, QCH], F32, tag="ops",
                                         name="ops")
                    pT, off = pTs.pop((j, qc, kc))
                    nc.tensor.matmul(o_ps[:, off:QCH], v_sb[:, kc, j, :],
                                     pT[:, off:QCH],
                                     start=(kc == 0), stop=(kc == kmax - 1))
                    # lookahead AFTER the PV: each finalize's o_sb drain
                    # enters the queues ahead of the next S/exp/tri burst
                    lk = LOOKAHEAD + (1 if i >= len(blocks) - 12 else 0)
                    for t in (i + LOOKAHEAD, i + lk):
                        if t < len(blocks) and blocks[t] not in pTs:
                            pTs[blocks[t]] = emit_s(*blocks[t])
                    if kc != kmax - 1:
                        continue
                    # ---- finalize q-chunk qc (bf16, per-128-row pipeline) ----
                    last = (h == NPASS - 1 and i == len(blocks) - 1)
                    o_sb = opool.tile([HD + 1, QCH], BF16, tag="o_sb",
                                      name="o_sb")
                    nc.vector.tensor_copy(o_sb, o_ps)
                    NB = QCH // P
                    tpo = ps_t.tile([P, NB * (HD + 2)], BF16, tag="tps",
                                    name="tpo")
                    nat3 = opool.tile([P, NB, REP * HD], BF16, tag="nat",
                                      name="nat")
                    rec = opool.tile([P, NB], F32, tag="rec", name="rec")
                    for blk in range(NB):
                        nc.tensor.transpose(
                            tpo[:, blk * (HD + 2):blk * (HD + 2) + HD + 1],
                            o_sb[:, blk * P:(blk + 1) * P],
                            ident[0:HD + 1, 0:HD + 1])
                    dma_eng = ([nc.sync, nc.scalar] if last else
                               [nc.gpsimd, nc.sync])
                    # one strided reciprocal covers all 4 denominators; the
                    # normalize runs as two 2-block ops so each output DMA
                    # launches as soon as its half is ready
                    tpb = tpo.rearrange("p (b c) -> p b c", b=NB)
                    nc.vector.reciprocal(rec, tpb[:, :, HD])
                    for half in range(2):
                        b0 = 2 * half
                        nc.vector.tensor_tensor(
                            nat3[:, b0:b0 + 2, :].rearrange(
                                "p b (r c) -> p b r c", r=REP),
                            tpb[:, b0:b0 + 2, None, 0:HD].to_broadcast(
                                (P, 2, REP, HD)),
                            rec[:, b0:b0 + 2, None, None].to_broadcast(
                                (P, 2, REP, HD)), mult)
                        row0 = qc * QCH + b0 * P
                        dst = out_d[row0:row0 + 2 * P,
                                    lg * D:(lg + 1) * D].rearrange(
                            "(b t) c -> t b c", b=2)
                        dma_eng[half].dma_start(
                            dst, nat3[:, b0:b0 + 2, :])

    nc.compile()
    return nc


_NC_CACHE = {}


def _get_nc(use_bias=False):
    if use_bias not in _NC_CACHE:
        _NC_CACHE[use_bias] = _build_nc(use_bias)
    return _NC_CACHE[use_bias]


def _host_inputs(x, Wq, bq, Wk, bk, Wv, bv):
    import ml_dtypes
    bf16 = ml_dtypes.bfloat16

    j = np.arange(D // 2)
    angles = 1.0 / (THETA ** ((2.0 * j) / D))
    th = np.arange(T, dtype=np.float64)[:, None] * angles[None, :]
    cosn = np.cos(th).astype(bf16)
    sinn = np.sin(th).astype(bf16)

    perm_q = np.concatenate([np.arange(0, D, 2), np.arange(1, D, 2)])
    eo = np.concatenate([np.arange(0, HD, 2), np.arange(1, HD, 2)])

    Wq = np.asarray(Wq, np.float32)
    Wk = np.asarray(Wk, np.float32)
    Wv = np.asarray(Wv, np.float32)
    bq = np.asarray(bq, np.float32)
    bk = np.asarray(bk, np.float32)
    bv = np.asarray(bv, np.float32)
    x = np.asarray(x, np.float32)

    in_maps = []
    for c in range(8):
        b, gh = divmod(c, 2)
        wblocks, bblocks = [], []
        for hh in range(NPASS):
            gs = [gh * GPC + GPP * hh + jj for jj in range(GPP)]
            for g in gs:
                wblocks.append(Wq[:, g * D:(g + 1) * D][:, perm_q])
                bblocks.append(bq[g * D:(g + 1) * D][perm_q])
            for g in gs:
                wblocks.append(Wk[:, g * HD:(g + 1) * HD][:, eo])
                bblocks.append(bk[g * HD:(g + 1) * HD][eo])
            for g in gs:
                wblocks.append(Wv[:, g * HD:(g + 1) * HD])
                bblocks.append(bv[g * HD:(g + 1) * HD])
        w_core = np.ascontiguousarray(
            np.concatenate(wblocks, axis=1)).astype(bf16)
        b_core = np.concatenate(bblocks)[None, :].astype(bf16)
        b_core = np.ascontiguousarray(b_core)
        in_maps.append({
            "xt": np.ascontiguousarray(x[b].T).astype(bf16),
            "w": w_core,
            "bias": b_core,
            "cos": cosn,
            "sin": sinn,
        })
    return in_maps


def kernel(x, Wq, bq, Wk, bk, Wv, bv, _trace=False, _trace_kwargs=None):
    in_maps = _host_inputs(x, Wq, bq, Wk, bk, Wv, bv)
    use_bias = bool(max(np.abs(np.asarray(b)).max() for b in (bq, bk, bv)) > 0)
    nc = _get_nc(use_bias)
    res = run_bass_kernel_spmd(nc, in_maps, core_ids=list(range(8)),
                               trace=_trace, **(_trace_kwargs or {}))
    out = np.empty((B, T, E), np.float32)
    for c in range(8):
        b, gh = divmod(c, 2)
        out[b, :, gh * GPC * D:(gh + 1) * GPC * D] = \
            res.results[c]["out"].astype(np.float32)
    if _trace:
        return out, res
    return out
